# revision 1
# baseline (speedup 1.0000x reference)
"""Causal self-attention Trainium2 kernel (B=2, T=2048, C=1024, H=16).

Sharding: tensor-parallel over heads (4-way) x data-parallel over batch (2-way)
= 8 cores. Core c handles batch b = c//4 and heads [4*(c%4), 4*(c%4)+4).
Each core computes x @ W_attn for its head slice, causal attention for its 4
heads, and a partial y @ W_proj over its 256 channels. The host sums the 4
partials per batch element (no device collectives).

Matmul operands are fp16 (full-rate PE; fp32 matmul is 4x slower). All PSUM
accumulation is fp32. Weights are host-cast to fp16; activations are cast at
the PSUM->SBUF copy that follows each producing matmul.

Layouts (per core, b fixed):
  xT   [c, t]   : 8 c-tiles of [128, 2048]  (DMA-xbar transposed from x)
  qT/kT[d', t]  : per head-pair hp, [128, 2048]; partitions 0-63 = head 2hp,
                  64-127 = head 2hp+1
  v    [t, d']  : [128 (t in s-tile), 16 s-tiles, 256 (4 local heads x 64)]
  S^T  [s, t]   : scores transposed; softmax sum over s via concurrent
                  ones-column matmuls; no max-subtraction (|S| <~ 3).
  y2 PSUM       : bank0 = y'_a (parts 0-63) + y'_b (parts 64-127),
                  bank1 = denom_a (part 0) + denom_b (part 64)
"""

import sys

if "/opt/trn_rl_repo" not in sys.path:
    sys.path.insert(0, "/opt/trn_rl_repo")

import numpy as np

import concourse.bass as bass
import concourse.bacc as bacc
import concourse.mybir as mybir
import concourse.tile as tile
from concourse.bass_utils import run_bass_kernel_spmd

F32 = mybir.dt.float32
F16 = mybir.dt.float16

B, T, C = 2, 2048, 1024
NH = 16              # total heads
D = 64               # head dim
N_CORES = 8
HG = 4               # heads per core
FC = HG * D          # 256 f-columns per core per q/k/v
CT = C // 128        # 8 c-tiles
TT = T // 128        # 16 t-tiles / s-tiles
TB = T // 512        # 4 t-blocks
NEG = -1.0e10
SCALE = 1.0 / 8.0    # 1/sqrt(D)


def build():
    nc = bacc.Bacc("TRN2", target_bir_lowering=False, debug=False,
                   num_devices=N_CORES)
    x_d = nc.dram_tensor("x", [T, C], F16, kind="ExternalInput").ap()
    wq_d = nc.dram_tensor("wq", [C, FC], F16, kind="ExternalInput").ap()
    wk_d = nc.dram_tensor("wk", [C, FC], F16, kind="ExternalInput").ap()
    wv_d = nc.dram_tensor("wv", [C, FC], F16, kind="ExternalInput").ap()
    wp_d = nc.dram_tensor("wp", [FC, C], F16, kind="ExternalInput").ap()
    out_d = nc.dram_tensor("out", [T, C], F32, kind="ExternalOutput").ap()

    with tile.TileContext(nc) as tc:
        body(tc, x_d, wq_d, wk_d, wv_d, wp_d, out_d)
    nc.compile()
    return nc


def body(tc, x_d, wq_d, wk_d, wv_d, wp_d, out_d):
    nc = tc.nc
    Exp = mybir.ActivationFunctionType.Exp

    with (
        tc.tile_pool(name="sb", bufs=1) as sb,
        tc.tile_pool(name="ps", bufs=1, space="PSUM") as ps,
    ):
        # binary causal mask in S^T orientation: 1 where t - s >= 0 else 0
        mask = sb.tile([128, 128], F16)
        nc.gpsimd.memset(mask, 1.0)
        nc.gpsimd.affine_select(
            out=mask, in_=mask, compare_op=mybir.AluOpType.is_ge,
            fill=0.0, base=0, pattern=[[1, 128]], channel_multiplier=-1)
        ones = sb.tile([128, 64], F16)
        nc.gpsimd.memset(ones, 1.0)


        wq_sb = sb.tile([128, CT, FC], F16)
        wk_sb = sb.tile([128, CT, FC], F16)
        wv_sb = sb.tile([128, CT, FC], F16)
        wp_sb = sb.tile([128, 2, C], F16)          # [c'(128), hp, n]
        nc.sync.dma_start(wv_sb, wv_d.rearrange("(ct p) f -> p ct f", p=128))
        xT = sb.tile([128, CT, T], F16)            # [c_local, ct, t]
        qT = sb.tile([128, 2, T], F16)             # [d', hp, t]
        kT = sb.tile([128, 2, T], F16)
        vp = sb.tile([128, TT, FC], F16)           # [t_in_tile, s_tile, lh*64]
        yT = sb.tile([128, 2, T], F16)             # [d', hp, t]

        # transpose x into xT with the DMA xbar (fp16, 2-byte dtype),
        # one [512, 128] -> [128, 512] transpose per (t-block, c-tile).
        # q/k weights load between t-block 0 (which gates the v matmuls)
        # and the rest of the stream.
        nc.sync.dma_start(wq_sb, wq_d.rearrange("(ct p) f -> p ct f", p=128))
        for ci in range(CT):
            nc.sync.dma_start(
                xT[:, ci, 0:512], x_d[0:512, ci * 128:(ci + 1) * 128],
                transpose=True)
        nc.sync.dma_start(wk_sb, wk_d.rearrange("(ct p) f -> p ct f", p=128))
        for tb in range(1, TB):
            for ci in range(CT):
                nc.sync.dma_start(
                    xT[:, ci, tb * 512:(tb + 1) * 512],
                    x_d[tb * 512:(tb + 1) * 512, ci * 128:(ci + 1) * 128],
                    transpose=True)
        # proj weights are only needed at the end; load them after the
        # latency-critical x transpose stream
        nc.sync.dma_start(wp_sb, wp_d.rearrange("(hp p) n -> p hp n", p=128))

        # Everything except the y' accumulator shares one 3-slot
        # [128,1024] PSUM rotation, and work is emitted "streamed": each
        # attention t-block right after the qkv groups it depends on, so
        # the ACT-bound exp pipeline overlaps qkv's PE-bound matmuls.
        def sp_tile(shape, name, dtype=F32):
            return ps.tile(shape, dtype, tag="sp", name=name, bufs=3)

        # HAM warmup: the PE is DMA-blocked for ~7us at startup, so the
        # first real matmuls would run at the throttled 1.2 GHz clock.
        # ~5us of dummy matmuls on constants (dependency-free) un-throttle
        # the array while the x transposes stream in.
        warm = sp_tile([64, 128], "warm")
        for _ in range(32):
            nc.tensor.matmul(warm, lhsT=ones, rhs=mask, start=True,
                             stop=True)
        # prefetch the exp spline-table load (~2.7us) into the startup
        # window too, so the first real exp doesn't pay it
        warm_e = sb.tile([1, 64], F16)
        nc.scalar.activation(warm_e, ones[0:1, :], Exp, scale=SCALE)

        def emit_v(tt):
            v_ps = sp_tile([128, FC], "v_ps")
            for ci in range(CT):
                nc.tensor.matmul(
                    v_ps,
                    lhsT=xT[:, ci, tt * 128:(tt + 1) * 128],
                    rhs=wv_sb[:, ci, :],
                    start=(ci == 0), stop=(ci == CT - 1))
            nc.vector.tensor_copy(vp[:, tt, :], v_ps)

        def emit_qk_group(hp, w_sb, dst, tb):
            qk_ps = sp_tile([128, 512], "qk_ps")
            for ci in range(CT):
                nc.tensor.matmul(
                    qk_ps,
                    lhsT=w_sb[:, ci, hp * 128:(hp + 1) * 128],
                    rhs=xT[:, ci, tb * 512:(tb + 1) * 512],
                    start=(ci == 0), stop=(ci == CT - 1))
            nc.vector.tensor_copy(
                dst[:, hp, tb * 512:(tb + 1) * 512], qk_ps)

        def emit_att_tb(hp, tb):
            lha, lhb = 2 * hp, 2 * hp + 1
            n_st = 4 * tb + 4              # causal: s-tiles 0..4tb+3
            # y2 bank0: y' both heads; bank1: denominators
            y2 = ps.tile([128, 1024], F32, tag="y2", bufs=1)
            for pair in range(n_st // 2):
                pts = []
                for hi in (0, 1):
                    sp = sp_tile([128, 1024], f"sp{hi}")
                    u0 = 0
                    for q2 in (0, 1):
                        si = 2 * pair + q2
                        kd = si - 4 * tb
                        col0 = 128 * kd if kd >= 0 else 0
                        if q2 == 0:
                            u0 = col0
                        nc.tensor.matmul(
                            sp[:, q2 * 512 + col0:(q2 + 1) * 512],
                            lhsT=kT[64 * hi:64 * hi + 64, hp,
                                    si * 128:(si + 1) * 128],
                            rhs=qT[64 * hi:64 * hi + 64, hp,
                                   tb * 512 + col0:(tb + 1) * 512],
                            start=True, stop=True)
                    pt = sb.tile([128, 1024], F16, tag=f"pt{hi}",
                                 name=f"pt{hi}", bufs=3)
                    nc.scalar.activation(pt[:, u0:], sp[:, u0:], Exp,
                                         scale=SCALE)
                    # zero the invalid triangle of diagonal squares after
                    # exp (exp * 0 == masked exp, and it keeps the DVE off
                    # the S -> exp critical path)
                    for q2 in (0, 1):
                        si = 2 * pair + q2
                        kd = si - 4 * tb
                        if kd >= 0:
                            col0 = 128 * kd
                            psl = pt[:, q2 * 512 + col0:
                                     q2 * 512 + col0 + 128]
                            nc.vector.tensor_mul(psl, psl, mask)
                    pts.append(pt)
                # PV + denominator accumulation; on the final s-tile the
                # denominators go first so the reciprocals overlap the
                # last V matmuls
                for q2 in (0, 1):
                    si = 2 * pair + q2
                    kd = si - 4 * tb
                    col0 = 128 * kd if kd >= 0 else 0
                    first, last = si == 0, si == n_st - 1
                    pa, pb = pts
                    vmm = [
                        (y2[0:64, col0:512],
                         vp[:, si, 64 * lha:64 * lha + 64], pa),
                        (y2[64:128, col0:512],
                         vp[:, si, 64 * lhb:64 * lhb + 64], pb),
                    ]
                    dmm = [
                        (y2[0:1, 512 + col0:1024], ones[:, 0:1], pa),
                        (y2[64:65, 512 + col0:1024], ones[:, 0:1], pb),
                    ]
                    groups = dmm + vmm if last else vmm + dmm
                    for out_ap, w_ap, p_ap in groups:
                        nc.tensor.matmul(
                            out_ap, lhsT=w_ap,
                            rhs=p_ap[:, q2 * 512 + col0:(q2 + 1) * 512],
                            start=first, stop=last)
            # normalize: yT = y' * (1/denom) broadcast across partitions
            rcp = sb.tile([128, 512], F16, tag="rcp", bufs=3)
            with nc.allow_low_precision(reason="softmax denom recip f16"):
                nc.vector.reciprocal(rcp[0:1, :], y2[0:1, 512:1024])
                nc.vector.reciprocal(rcp[64:65, :], y2[64:65, 512:1024])
            # the denominator bank of y2 is dead after the reciprocals;
            # broadcast into it instead of taking a slot from the rotation
            nc.tensor.matmul(y2[0:64, 512:1024], lhsT=ones[0:1, :],
                             rhs=rcp[0:1, :], start=True, stop=True)
            nc.tensor.matmul(y2[64:128, 512:1024], lhsT=ones[64:65, :],
                             rhs=rcp[64:65, :], start=True, stop=True)
            bp_sb = sb.tile([128, 512], F32, tag="bps", bufs=3)
            nc.vector.tensor_copy(bp_sb, y2[:, 512:1024])
            nc.vector.tensor_mul(
                yT[0:64, hp, tb * 512:(tb + 1) * 512],
                y2[0:64, 0:512], bp_sb[0:64, :])
            nc.vector.tensor_mul(
                yT[64:128, hp, tb * 512:(tb + 1) * 512],
                y2[64:128, 0:512], bp_sb[64:128, :])

        def emit_proj(tt):
            pj = sp_tile([128, 1024], "pj")
            for hp in range(2):
                for nb in range(2):
                    nc.tensor.matmul(
                        pj[:, nb * 512:(nb + 1) * 512],
                        lhsT=yT[:, hp, tt * 128:(tt + 1) * 128],
                        rhs=wp_sb[:, hp, nb * 512:(nb + 1) * 512],
                        start=(hp == 0), stop=(hp == 1))
            ob = sb.tile([128, 1024], F32, tag="ob", bufs=4)
            nc.scalar.copy(ob[:, 0:512], pj[:, 0:512])
            nc.vector.tensor_copy(ob[:, 512:1024], pj[:, 512:1024])
            nc.sync.dma_start(out_d[tt * 128:(tt + 1) * 128, :], ob)

        # ---- streamed emission ----
        for tb in range(TB):
            for tt in range(4 * tb, 4 * tb + 4):
                emit_v(tt)
            emit_qk_group(0, wq_sb, qT, tb)
            emit_qk_group(0, wk_sb, kT, tb)
            emit_att_tb(0, tb)
        # hp1's first q/k groups fill hp0's final normalize tail
        emit_qk_group(1, wq_sb, qT, 0)
        emit_qk_group(1, wk_sb, kT, 0)
        for tb in range(TB):
            emit_att_tb(1, tb)
            # prefetch the next t-block's q/k before this block's proj so
            # the next attention block starts without waiting behind proj
            if tb + 1 < TB:
                emit_qk_group(1, wq_sb, qT, tb + 1)
                emit_qk_group(1, wk_sb, kT, tb + 1)
            for tt in range(4 * tb, 4 * tb + 4):
                emit_proj(tt)


_NC_CACHE = None


def _get_nc():
    global _NC_CACHE
    if _NC_CACHE is None:
        _NC_CACHE = build()
    return _NC_CACHE


def _in_maps(x, W_attn, W_proj):
    x16 = x.astype(np.float16)
    wa16 = W_attn.astype(np.float16)
    wp16 = W_proj.astype(np.float16)
    maps = []
    for core in range(N_CORES):
        b, g = core // 4, core % 4
        f0 = FC * g
        maps.append({
            "x": np.ascontiguousarray(x16[b]),
            "wq": np.ascontiguousarray(wa16[:, f0:f0 + FC]),
            "wk": np.ascontiguousarray(wa16[:, C + f0:C + f0 + FC]),
            "wv": np.ascontiguousarray(wa16[:, 2 * C + f0:2 * C + f0 + FC]),
            "wp": np.ascontiguousarray(wp16[f0:f0 + FC, :]),
        })
    return maps


def run(x, W_attn, W_proj, trace=False, **kwargs):
    nc = _get_nc()
    res = run_bass_kernel_spmd(nc, _in_maps(x, W_attn, W_proj),
                               core_ids=list(range(N_CORES)),
                               trace=trace, **kwargs)
    out = np.zeros((B, T, C), dtype=np.float32)
    for core in range(N_CORES):
        out[core // 4] += res.results[core]["out"]
    return out, res


def kernel(x, W_attn, W_proj):
    x = np.asarray(x, dtype=np.float32)
    W_attn = np.asarray(W_attn, dtype=np.float32)
    W_proj = np.asarray(W_proj, dtype=np.float32)
    out, _ = run(x, W_attn, W_proj, trace=False)
    return out



# revision 4
# speedup vs baseline: 1.2585x; 1.2585x over previous
"""Causal self-attention Trainium2 kernel (B=2, T=2048, C=1024, H=16).

Sharding: tensor-parallel over heads (4-way) x data-parallel over batch (2-way)
= 8 cores. Core c handles batch b = c//4 and heads [4*(c%4), 4*(c%4)+4).
Each core computes x @ W_attn for its head slice, causal attention for its 4
heads, and a partial y @ W_proj over its 256 channels. The host sums the 4
partials per batch element (no device collectives).

Matmul operands are fp16 (full-rate PE; fp32 matmul is 4x slower). All PSUM
accumulation is fp32. Weights are host-cast to fp16; activations are cast at
the PSUM->SBUF copy that follows each producing matmul.

Layouts (per core, b fixed):
  xT   [c, t]    : 8 c-tiles of [128, 2048]  (DMA-xbar transposed from x)
  qT/kT[d', t]   : per head-pair hp, [128, 2048]; partitions 0-63 = head 2hp,
                   64-127 = head 2hp+1
  vp   [s, h, d']: [128 (s in tile), 16 s-tiles, 4 heads, 65]; col 64 is a
                   ones-column so PV emits the softmax denominator for free
  sp   [s, hi, t]: scores for one s-tile, both heads of hp, PSUM [128,2,512]
  pt             : exp(sp) in SBUF, same layout, per (hp, s-tile)
  y2   [t, h, 65]: PV output t-major; col 64 = denominator. Normalize is a
                   per-partition tensor_scalar multiply, then a PE transpose
                   back to [c', t] for the projection.
"""

import sys

if "/opt/trn_rl_repo" not in sys.path:
    sys.path.insert(0, "/opt/trn_rl_repo")

import numpy as np

import concourse.bass as bass
import concourse.bacc as bacc
import concourse.mybir as mybir
import concourse.tile as tile
from concourse.bass_utils import run_bass_kernel_spmd

F32 = mybir.dt.float32
F16 = mybir.dt.float16

B, T, C = 2, 2048, 1024
NH = 16              # total heads
D = 64               # head dim
N_CORES = 8
HG = 4               # heads per core
FC = HG * D          # 256 f-columns per core per q/k/v
CT = C // 128        # 8 c-tiles
TT = T // 128        # 16 t-tiles / s-tiles
TB = T // 512        # 4 t-blocks
SCALE = 1.0 / 8.0    # 1/sqrt(D)


def build():
    nc = bacc.Bacc("TRN2", target_bir_lowering=False, debug=False,
                   num_devices=N_CORES)
    x_d = nc.dram_tensor("x", [T, C], F16, kind="ExternalInput").ap()
    wq_d = nc.dram_tensor("wq", [C, FC], F16, kind="ExternalInput").ap()
    wk_d = nc.dram_tensor("wk", [C, FC], F16, kind="ExternalInput").ap()
    wv_d = nc.dram_tensor("wv", [C, FC], F16, kind="ExternalInput").ap()
    wp_d = nc.dram_tensor("wp", [FC, C], F16, kind="ExternalInput").ap()
    out_d = nc.dram_tensor("out", [T, C], F32, kind="ExternalOutput").ap()

    with tile.TileContext(nc) as tc:
        body(tc, x_d, wq_d, wk_d, wv_d, wp_d, out_d)
    nc.compile()
    return nc


def body(tc, x_d, wq_d, wk_d, wv_d, wp_d, out_d):
    nc = tc.nc
    Exp = mybir.ActivationFunctionType.Exp

    with (
        tc.tile_pool(name="sb", bufs=1) as sb,
        tc.tile_pool(name="ps", bufs=1, space="PSUM") as ps,
    ):
        # binary causal mask in S^T orientation: 1 where t - s >= 0 else 0
        mask = sb.tile([128, 128], F16)
        nc.gpsimd.memset(mask, 1.0)
        nc.gpsimd.affine_select(
            out=mask, in_=mask, compare_op=mybir.AluOpType.is_ge,
            fill=0.0, base=0, pattern=[[1, 128]], channel_multiplier=-1)
        # identity for PE transposes: diagonal of the two triangle selects
        ident = sb.tile([128, 128], F16)
        nc.gpsimd.memset(ident, 1.0)
        nc.gpsimd.affine_select(
            out=ident, in_=ident, compare_op=mybir.AluOpType.is_ge,
            fill=0.0, base=0, pattern=[[1, 128]], channel_multiplier=-1)
        nc.gpsimd.affine_select(
            out=ident, in_=ident, compare_op=mybir.AluOpType.is_ge,
            fill=0.0, base=0, pattern=[[-1, 128]], channel_multiplier=1)
        ones = sb.tile([128, 64], F16)
        nc.gpsimd.memset(ones, 1.0)

        wq_sb = sb.tile([128, CT, FC], F16)
        wk_sb = sb.tile([128, CT, FC], F16)
        wv_sb = sb.tile([128, CT, FC], F16)
        wp_sb = sb.tile([128, 2, C], F16)          # [c'(128), hp, n]
        nc.sync.dma_start(wv_sb, wv_d.rearrange("(ct p) f -> p ct f", p=128))
        xT = sb.tile([128, CT, T], F16)            # [c_local, ct, t]
        qT = sb.tile([128, 2, T], F16)             # [d', hp, t]
        kT = sb.tile([128, 2, T], F16)
        vp = sb.tile([128, TT, HG, 65], F16)       # [s_in_tile, s_tile, h, d'+1]
        pt0 = sb.tile([128, TT, 2, 512], F16)      # [s, s_tile, hi, t_in_tb]
        pt1 = sb.tile([128, TT, 2, 512], F16)
        pts = (pt0, pt1)
        yT = sb.tile([128, 2, T], F16)             # [c', hp, t]
        nc.gpsimd.memset(vp[:, :, :, 64:65], 1.0)  # PV denominator column

        # transpose x into xT with the DMA xbar (fp16, 2-byte dtype),
        # one [512, 128] -> [128, 512] transpose per (t-block, c-tile).
        nc.sync.dma_start(wq_sb, wq_d.rearrange("(ct p) f -> p ct f", p=128))
        for ci in range(CT):
            nc.sync.dma_start(
                xT[:, ci, 0:512], x_d[0:512, ci * 128:(ci + 1) * 128],
                transpose=True)
        nc.sync.dma_start(wk_sb, wk_d.rearrange("(ct p) f -> p ct f", p=128))
        for tb in range(1, TB):
            for ci in range(CT):
                nc.sync.dma_start(
                    xT[:, ci, tb * 512:(tb + 1) * 512],
                    x_d[tb * 512:(tb + 1) * 512, ci * 128:(ci + 1) * 128],
                    transpose=True)
        # proj weights are only needed at the end; load them after the
        # latency-critical x transpose stream
        nc.sync.dma_start(wp_sb, wp_d.rearrange("(hp p) n -> p hp n", p=128))

        # PSUM: sp 2x2 banks + qv 2x1 + ar 2x1 = 8 banks
        def sp_tile():
            return ps.tile([128, 2, 512], F32, tag="sp", name="sp", bufs=2)

        def qv_tile(name):
            return ps.tile([128, 512], F32, tag="qv", name=name, bufs=2)

        def ar_tile():
            return ps.tile([128, 512], F32, tag="ar", name="ar", bufs=2)

        # HAM warmup: the PE is DMA-blocked for ~7us at startup, so the
        # first real matmuls would run at the throttled clock. ~4us of
        # dummy matmuls on constants un-throttle the array while the x
        # transposes stream in; also prefetch the exp spline table.
        for _ in range(16):
            warm = qv_tile("warm")
            nc.tensor.matmul(warm[0:64, 0:128], lhsT=ones, rhs=mask,
                             start=True, stop=True)
            nc.tensor.matmul(warm[0:64, 128:256], lhsT=ones, rhs=mask,
                             start=True, stop=True)
        warm_e = sb.tile([1, 64], F16)
        nc.scalar.activation(warm_e, ones[0:1, :], Exp, scale=SCALE)

        def emit_v(tt):
            v_ps = qv_tile("v_ps")
            for ci in range(CT):
                nc.tensor.matmul(
                    v_ps[:, 0:256],
                    lhsT=xT[:, ci, tt * 128:(tt + 1) * 128],
                    rhs=wv_sb[:, ci, :],
                    start=(ci == 0), stop=(ci == CT - 1))
            nc.vector.tensor_copy(
                vp[:, tt, :, 0:64],
                v_ps[:, 0:256].rearrange("p (h d) -> p h d", h=HG))

        def emit_qk(hp, w_sb, dst, tb):
            qk_ps = qv_tile("qk_ps")
            for ci in range(CT):
                nc.tensor.matmul(
                    qk_ps,
                    lhsT=w_sb[:, ci, hp * 128:(hp + 1) * 128],
                    rhs=xT[:, ci, tb * 512:(tb + 1) * 512],
                    start=(ci == 0), stop=(ci == CT - 1))
            nc.vector.tensor_copy(
                dst[:, hp, tb * 512:(tb + 1) * 512], qk_ps)

        def emit_qk_si(hp, tb, si):
            # scores S^T for one s-tile, both heads of pair hp, then one
            # exp covering both heads with the causal prefix trimmed
            kd = si - 4 * tb
            col0 = 128 * kd if kd > 0 else 0
            sp = sp_tile()
            for hi in (0, 1):
                nc.tensor.matmul(
                    sp[:, hi, col0:512],
                    lhsT=kT[64 * hi:64 * hi + 64, hp,
                            si * 128:(si + 1) * 128],
                    rhs=qT[64 * hi:64 * hi + 64, hp,
                           tb * 512 + col0:(tb + 1) * 512],
                    start=True, stop=True)
            pt = pts[hp]
            nc.scalar.activation(pt[:, si, :, col0:512], sp[:, :, col0:512],
                                 Exp, scale=SCALE)
            if kd >= 0:
                # zero the invalid triangle of the diagonal square after
                # exp (exp * 0 == masked exp, off the S -> exp hot path)
                for hi in (0, 1):
                    psl = pt[:, si, hi, col0:col0 + 128]
                    nc.vector.tensor_mul(psl, psl, mask)

        def emit_pv_tt(tb, tt):
            ttl = tt - 4 * tb
            slot = ar_tile()
            y2 = slot[:, 0:260].rearrange("p (h x) -> p h x", h=HG)
            for h in range(HG):
                hp, hi = h // 2, h % 2
                pt = pts[hp]
                for si in range(tt + 1):
                    nc.tensor.matmul(
                        y2[:, h, :],
                        lhsT=pt[:, si, hi, ttl * 128:(ttl + 1) * 128],
                        rhs=vp[:, si, h, :],
                        start=(si == 0), stop=(si == tt))
            rcp = sb.tile([128, HG], F32, tag="rcp", name="rcp", bufs=3)
            nc.vector.reciprocal(rcp, y2[:, :, 64])
            y_sb = sb.tile([128, FC], F16, tag="ysb", name="y_sb", bufs=3)
            for h in range(HG):
                nc.vector.tensor_scalar_mul(
                    y_sb[:, h * 64:(h + 1) * 64], y2[:, h, 0:64],
                    rcp[:, h:h + 1])
            # PE transpose [t, c'] -> [c', t] for the projection lhsT
            yt = slot[:, 288:416].bitcast(F16)     # [128, 256] f16
            for ch in range(2):
                nc.tensor.transpose(
                    yt[:, ch * 128:(ch + 1) * 128],
                    y_sb[:, ch * 128:(ch + 1) * 128], ident)
            nc.vector.tensor_copy(
                yT[:, :, tt * 128:(tt + 1) * 128],
                yt.rearrange("p (hp t) -> p hp t", hp=2))

        def emit_proj(tt):
            ob = sb.tile([128, C], F32, tag="ob", name="ob", bufs=3)
            for nb in range(2):
                pj = qv_tile("pj")
                for hp in range(2):
                    nc.tensor.matmul(
                        pj,
                        lhsT=yT[:, hp, tt * 128:(tt + 1) * 128],
                        rhs=wp_sb[:, hp, nb * 512:(nb + 1) * 512],
                        start=(hp == 0), stop=(hp == 1))
                if nb == 0:
                    nc.scalar.copy(ob[:, 0:512], pj)
                else:
                    nc.vector.tensor_copy(ob[:, 512:1024], pj)
            nc.sync.dma_start(out_d[tt * 128:(tt + 1) * 128, :], ob)

        # ---- streamed emission ----
        # qkv for t-block 0 up front; later blocks' qkv drip-fed into the
        # attention stream so the PE always has work while exp runs
        for tt in range(4):
            emit_v(tt)
        for hp in range(2):
            emit_qk(hp, wq_sb, qT, 0)
            emit_qk(hp, wk_sb, kT, 0)

        def deferred_for(tb):
            if tb >= TB:
                return []
            work = []
            for i in range(4):
                work.append(lambda tt=4 * tb + i: emit_v(tt))
                hp, w_sb, dst = ((0, wq_sb, qT), (0, wk_sb, kT),
                                 (1, wq_sb, qT), (1, wk_sb, kT))[i]
                work.append(lambda hp=hp, w=w_sb, d=dst, tb=tb:
                            emit_qk(hp, w, d, tb))
            return work

        prev_proj = []   # proj emission lags PV by one t-tile
        for tb in range(TB):
            work = deferred_for(tb + 1)
            n_si = 4 * tb + 4
            done = 0
            for si in range(n_si):
                for hp in range(2):
                    emit_qk_si(hp, tb, si)
                want = (si + 1) * len(work) // n_si
                while done < want:
                    work[done]()
                    done += 1
                if si >= 4 * tb:
                    tt = si
                    emit_pv_tt(tb, tt)
                    for p in prev_proj:
                        p()
                    prev_proj = [lambda tt=tt: emit_proj(tt)]
        for p in prev_proj:
            p()


_NC_CACHE = None


def _get_nc():
    global _NC_CACHE
    if _NC_CACHE is None:
        _NC_CACHE = build()
    return _NC_CACHE


def _in_maps(x, W_attn, W_proj):
    x16 = x.astype(np.float16)
    wa16 = W_attn.astype(np.float16)
    wp16 = W_proj.astype(np.float16)
    maps = []
    for core in range(N_CORES):
        b, g = core // 4, core % 4
        f0 = FC * g
        maps.append({
            "x": np.ascontiguousarray(x16[b]),
            "wq": np.ascontiguousarray(wa16[:, f0:f0 + FC]),
            "wk": np.ascontiguousarray(wa16[:, C + f0:C + f0 + FC]),
            "wv": np.ascontiguousarray(wa16[:, 2 * C + f0:2 * C + f0 + FC]),
            "wp": np.ascontiguousarray(wp16[f0:f0 + FC, :]),
        })
    return maps


def run(x, W_attn, W_proj, trace=False, **kwargs):
    nc = _get_nc()
    res = run_bass_kernel_spmd(nc, _in_maps(x, W_attn, W_proj),
                               core_ids=list(range(N_CORES)),
                               trace=trace, **kwargs)
    out = np.zeros((B, T, C), dtype=np.float32)
    for core in range(N_CORES):
        out[core // 4] += res.results[core]["out"]
    return out, res


def kernel(x, W_attn, W_proj):
    x = np.asarray(x, dtype=np.float32)
    W_attn = np.asarray(W_attn, dtype=np.float32)
    W_proj = np.asarray(W_proj, dtype=np.float32)
    out, _ = run(x, W_attn, W_proj, trace=False)
    return out
